# revision 33
# baseline (speedup 1.0000x reference)
"""Trainium2 Bass kernel: single-head causal attention (B=4, S=2048, D=1024).

reference:
  K = Xk @ WK; Q = Xq @ WQ; V = Xv @ WV          [B,S,D] @ [D,D]
  out = softmax(causal(Q K^T / sqrt(D))) @ V      [B,S,D]

Sharding over 8 NeuronCores (one SPMD program, no collectives):
  core c -> (batch b = c//2, key-parity h = c%2)
  Data-parallel over batch; within each pair, flash-attention-style split
  over KEYS: core h owns the key tiles {t : t % 2 == h} (the host feeds
  Xk/Xv columns for those keys, packed densely).  Each core projects
  K^T/V only for its own keys (split for free), projects Q fully
  (duplicated - every query row needs scores against both key subsets),
  and computes the UNNORMALIZED partial attention
      o_h[q, :] = sum_{k in keys_h, k <= q} exp(s_qk) V[k, :]
      s_h[q]    = sum_{k in keys_h, k <= q} exp(s_qk)
  The host combines the pair: out = (o_0 + o_1) / (s_0 + s_1).

Causal masking stays SPMD-uniform: for query block gb only the LAST local
key tile ever needs masking, and the needed pattern depends only on
(parity of gb, core parity).  Two host-fed [128,128] mask tiles
(mask_even / mask_odd) cover all cases: triangular for the core that owns
the diagonal tile, all -1e30 for the phantom tile the other core computes
(its exp underflows to exactly 0, contributing nothing), zeros when the
last local tile is fully visible.

The host feeds X pre-transposed ([D, S] layout) and pre-cast to fp16.

Per-core pipeline (fp16 matmuls on the PE, fp32 PSUM + fp32 softmax):
  Phase A: load X^T chunks + W (fp16, direct DMA);
           Q^T full -> [e, q], K^T (own keys) -> [e, k_loc],
           V (own keys) -> [k_loc, e] natural.
  Phase B: per 128-query block gb (big/small interleave, ending small):
           scores = Q^T.T K^T over the <= 8 local key tiles,
           data-driven mask on the last tile, p = exp(scores/sqrt(D)) on
           ACT with fp32 row sums (no max-shift: scaled logits ~N(0,0.33)),
           PE-transpose p tiles, o = p^T.T @ V accumulated over key tiles
           into two 512-wide PSUM banks, DMA out unnormalized (fp16) plus
           row sums (fp32).
"""
import numpy as np

B, S, D = 4, 2048, 1024
P = 128
SB = S // P            # 16 key/query blocks
DC = D // P            # 8 contraction chunks of 128
EB = D // P            # 8 e-blocks of 128
KL = S // 2            # 1024: per-core local key count
KLB = KL // P          # 8 local key tiles
INV_SQRT_D = float(1.0 / np.sqrt(np.float64(D)))
NCORES = 8
MASKV = -1e30

_CACHE = {}


def _build_nc():
    import concourse.bacc as bacc
    import concourse.mybir as mybir
    import concourse.tile as tile
    from concourse.masks import make_identity
    from contextlib import ExitStack

    fp32 = mybir.dt.float32
    fp16 = mybir.dt.float16
    Exp = mybir.ActivationFunctionType.Exp
    Add = mybir.AluOpType.add
    X = mybir.AxisListType.X

    nc = bacc.Bacc("TRN2", target_bir_lowering=False, debug=False,
                   num_devices=NCORES)

    xk_d = nc.dram_tensor("xk", [D, KL], fp16, kind="ExternalInput")
    xv_d = nc.dram_tensor("xv", [D, KL], fp16, kind="ExternalInput")
    xq_d = nc.dram_tensor("xq", [D, S], fp16, kind="ExternalInput")
    wk_d = nc.dram_tensor("wk", [D, D], fp16, kind="ExternalInput")
    # Q projection is pair-deduped: each core computes its e-half of Q and
    # the halves are exchanged with a background AllGather
    wq_d = nc.dram_tensor("wq", [D, D // 2], fp16, kind="ExternalInput")
    wv_d = nc.dram_tensor("wv", [D, D], fp16, kind="ExternalInput")
    mke_d = nc.dram_tensor("mke", [P, P], fp32, kind="ExternalInput")
    mko_d = nc.dram_tensor("mko", [P, P], fp32, kind="ExternalInput")
    o_d = nc.dram_tensor("o", [S, D], fp16, kind="ExternalOutput")
    sums_d = nc.dram_tensor("sums", [P, SB], fp32, kind="ExternalOutput")

    copy_ctr = [0]

    with tile.TileContext(nc) as tc:
        with ExitStack() as top:
            persist = top.enter_context(tc.tile_pool(name="persist", bufs=1))
            qt_h = persist.tile([P, EB, S], fp16, name="qt_h")
            kt_h = persist.tile([P, EB, KL], fp16, name="kt_h")
            v_h = persist.tile([P, KLB, D], fp16, name="v_h")
            ident16 = persist.tile([P, P], fp16, name="ident16")
            mke = persist.tile([P, P], fp32, name="mke")
            mko = persist.tile([P, P], fp32, name="mko")
            sums_all = persist.tile([P, SB], fp32, name="sums_all")

            def alt_copy(dst, src):
                # round-robin PSUM->SBUF copies 2:1 between DVE and ACT
                i = copy_ctr[0]
                copy_ctr[0] += 1
                if i % 3 == 2:
                    nc.scalar.copy(dst, src)
                else:
                    nc.vector.tensor_copy(dst, src)

            # ---------------- Phase A: projections ----------------
            with ExitStack() as pa:
                wpool = pa.enter_context(tc.tile_pool(name="wpool", bufs=1))
                xtpool = pa.enter_context(tc.tile_pool(name="xtpool",
                                                       bufs=8))
                dram = pa.enter_context(
                    tc.tile_pool(name="dram", bufs=1, space="DRAM"))
                psA = pa.enter_context(
                    tc.tile_pool(name="psA", bufs=3, space="PSUM"))

                # masks/identity first: tiny DMAs + iota, no load deps
                make_identity(nc, ident16[:])
                nc.sync.dma_start(mke[:], mke_d[:, :])
                nc.sync.dma_start(mko[:], mko_d[:, :])

                qs = [nc.gpsimd, nc.sync, nc.scalar]

                def load_w(w_d, nm, qoff, ecols=D, q=None):
                    wh = wpool.tile([P, DC, ecols], fp16, name=nm, tag=nm)
                    src = w_d.rearrange("(c p) e -> p c e", p=P)
                    for i in range(4):
                        (q or qs[(qoff + i) % 3]).dma_start(
                            wh[:, 2 * i:2 * i + 2], src[:, 2 * i:2 * i + 2])
                    return wh

                def load_xt(x_d, ch, qoff, q=None):
                    """Columns [ch*512, (ch+1)*512) of x^T [D, cols] (fp16):
                    two parallel half-loads -> [P(d), DC, 512] fp16."""
                    xt = xtpool.tile([P, DC, 512], fp16, name="xt", tag="xt")
                    src = x_d.rearrange("(c p) s -> p c s", p=P)[
                        :, :, ch * 512:(ch + 1) * 512]
                    (q or qs[qoff % 3]).dma_start(xt[:, :4], src[:, :4])
                    (q or qs[(qoff + 1) % 3]).dma_start(xt[:, 4:], src[:, 4:])
                    return xt

                def blocker(region):
                    # a tiny sync-queue DMA reading an SBUF region that
                    # compute only writes later: its $S-wait stalls the
                    # sync queue, so prefetch triggers emitted after it
                    # genuinely hold off until that compute lands --
                    # without it every prefetch fires at t~8us and the
                    # critical first 3MB crawls behind 10MB of competing
                    # traffic
                    j = dram.tile([P, 64], fp16, name="junk")
                    nc.sync.dma_start(j[:], region)

                # Prefetch everything in consumption order: the 3 W's and
                # all 8 X chunks fit in wpool(3)+xtpool(8) simultaneously,
                # so no matmul after the first ever waits on a load.  The
                # first-needed 3MB (xq chunk 0 + wq) goes out first so the
                # first matmul group isn't stuck behind competing traffic.
                # critical first batch: xq chunk 0 on gpsimd+scalar, wq
                # quarters spread over all three queues
                xq_t = [load_xt(xq_d, 0, 0)]
                wq_h = load_w(wq_d, "w_q", 1, ecols=D // 2)

                # warm the PE's HAM clock gate with throwaway matmuls on the
                # identity tile while the first loads are still in flight
                for _ in range(6):
                    wps = psA.tile([P, 512], fp32, name="warm", tag="psa")
                    for j in range(8):
                        nc.tensor.matmul(wps[:, :P], ident16[:], ident16[:],
                                         start=(j == 0), stop=(j == 7))

                NEB = EB // 2  # local e-blocks of the deduped Q projection

                # Q-half projection, then K (own keys): [e, s] W-stationary.
                # Later loads are anchored behind earlier chunks' results
                # so the critical first 3MB has the DMA engines to itself;
                # everything else still arrives well ahead of use.
                wk_h = wv_h = None
                xk_t, xv_t = [], []
                for w_i, (dst, ncols, neb) in enumerate(
                        ((qt_h, S, NEB), (kt_h, KL, EB))):
                    w_h = wq_h if w_i == 0 else wk_h
                    for ch in range(ncols // 512):
                        xt = (xq_t if w_i == 0 else xk_t)[ch]
                        for eb in range(neb):
                            ps = psA.tile([P, 512], fp32, name="psa",
                                          tag="psa")
                            for dc in range(DC):
                                nc.tensor.matmul(
                                    ps[:],
                                    w_h[:, dc, eb * P:(eb + 1) * P],
                                    xt[:, dc, :],
                                    start=(dc == 0), stop=(dc == DC - 1))
                            alt_copy(dst[:, eb, ch * 512:ch * 512 + 512],
                                     ps[:])
                        if w_i == 0 and ch == 0:
                            # deferred prefetch of everything else, held
                            # back behind the first projected result
                            blocker(qt_h[:, 0, 0:64])
                            xq_t += [load_xt(xq_d, c, 0, q=nc.sync)
                                     for c in (1, 2, 3)]
                            wk_h = load_w(wk_d, "w_k", 0, q=nc.sync)
                            xk_t += [load_xt(xk_d, c, 0, q=nc.sync)
                                     for c in (0, 1)]
                            wv_h = load_w(wv_d, "w_v", 0, q=nc.sync)
                            xv_t += [load_xt(xv_d, c, 0, q=nc.sync)
                                     for c in (0, 1)]
                        if w_i == 0 and ch == 1:
                            # stage the first half of the Q e-half slab as
                            # soon as it is projected so the AllGather can
                            # trigger right after chunk 3 lands
                            in_b = dram.tile([P, NEB, S], fp16, name="q_in")
                            out_b = dram.tile([2, P, NEB, S], fp16,
                                              name="q_out")
                            nc.scalar.dma_start(in_b[:, :, :1024],
                                                qt_h[:, :NEB, :1024])
                        if w_i == 0 and ch == 3:
                            nc.scalar.dma_start(in_b[:, :, 1024:],
                                                qt_h[:, :NEB, 1024:])
                            # pair AllGather of the Q e-halves, hidden
                            # under the K and V projections; rank r's half
                            # lands at e-blocks [r*NEB, (r+1)*NEB) which
                            # matches how the host sliced WQ
                            nc.gpsimd.collective_compute(
                                "AllGather",
                                mybir.AluOpType.bypass,
                                replica_groups=[[0, 4], [1, 5],
                                                [2, 6], [3, 7]],
                                ins=[in_b.opt()],
                                outs=[out_b.opt()],
                            )
                            nc.sync.dma_start(qt_h[:, :NEB], out_b[0])
                            nc.sync.dma_start(qt_h[:, NEB:], out_b[1])

                # V projection (own keys, full e): out[k, e] X^T-stationary
                for ch in range(KL // 512):
                    xt = xv_t[ch]
                    for a in range(4):
                        for eh in range(2):
                            ps = psA.tile([P, 512], fp32, name="psa",
                                          tag="psa")
                            for dc in range(DC):
                                nc.tensor.matmul(
                                    ps[:],
                                    xt[:, dc, a * P:(a + 1) * P],
                                    wv_h[:, dc, eh * 512:eh * 512 + 512],
                                    start=(dc == 0), stop=(dc == DC - 1))
                            alt_copy(
                                v_h[:, ch * 4 + a,
                                    eh * 512:eh * 512 + 512], ps[:])

            # ---------------- Phase B: causal attention ----------------
            with ExitStack() as pb:
                ppool = pb.enter_context(tc.tile_pool(name="ppool", bufs=3))
                ptpool = pb.enter_context(tc.tile_pool(name="ptpool", bufs=3))
                smpool = pb.enter_context(tc.tile_pool(name="smpool", bufs=4))
                opool = pb.enter_context(tc.tile_pool(name="opool", bufs=4))
                psBs = pb.enter_context(
                    tc.tile_pool(name="psBs", bufs=2, space="PSUM"))
                psBt = pb.enter_context(
                    tc.tile_pool(name="psBt", bufs=2, space="PSUM"))
                psBo = pb.enter_context(
                    tc.tile_pool(name="psBo", bufs=2, space="PSUM"))

                # big/small interleave: every small block's serial softmax
                # chain hides behind a big block's matmul stream; end with
                # the smallest block so the tail is minimal
                order = []
                for i in range(SB // 2 - 1):
                    order.append(SB - 1 - i)
                    order.append(i + 1)
                order += [SB // 2, 0]
                for gb in order:
                    nk = gb // 2 + 1   # local key tiles (incl. any phantom)
                    kw = nk * P        # local visible key width
                    nch = (kw + 511) // 512
                    mask = mke if gb % 2 == 0 else mko

                    # streaming softmax without max-shift: scaled logits are
                    # ~N(0,0.33), so exp(s/sqrt(D)) is safely inside fp32
                    # range and softmax is shift-invariant. Each QK chunk
                    # goes straight from PSUM through exp; normalization
                    # happens on the host after the pair-combine.
                    p16 = ppool.tile([P, KL], fp16, name="p16", tag="p16")
                    sums4 = smpool.tile([P, 2], fp32, name="sums4",
                                        tag="sums4")
                    pt = ptpool.tile([P, KLB, P], fp16, name="pt", tag="pt")
                    for ci in range(nch):
                        c0 = ci * 512
                        w = min(512, kw - c0)
                        ps = psBs.tile([P, 512], fp32, name="ps_s", tag="ps_s")
                        for dc in range(DC):
                            nc.tensor.matmul(
                                ps[:, :w],
                                qt_h[:, dc, gb * P:(gb + 1) * P],
                                kt_h[:, dc, c0:c0 + w],
                                start=(dc == 0), stop=(dc == DC - 1))
                        if c0 + w == kw:
                            # data-driven causal mask on the last local tile
                            nc.vector.tensor_tensor(
                                ps[:, w - P:w], ps[:, w - P:w], mask[:], Add)
                        nc.scalar.activation(p16[:, c0:c0 + w], ps[:, :w],
                                             Exp, bias=0.0, scale=INV_SQRT_D,
                                             accum_out=sums4[:, ci:ci + 1])
                        for k0 in range(c0 // P, c0 // P + w // P, 4):
                            kn = min(4, nk - k0)
                            pst = psBt.tile([P, 512], fp16, name="ps_t",
                                            tag="ps_t")
                            for j in range(kn):
                                nc.tensor.transpose(
                                    pst[:, j * P:(j + 1) * P],
                                    p16[:, (k0 + j) * P:(k0 + j + 1) * P],
                                    ident16[:])
                            nc.vector.tensor_copy(
                                pt[:, k0:k0 + kn], pst[:, :kn * P])

                    nc.vector.tensor_reduce(sums_all[:, gb:gb + 1],
                                            sums4[:, :nch], X, Add)

                    pso = [psBo.tile([P, 512], fp32, name=f"ps_o{eh}",
                                     tag=f"ps_o{eh}") for eh in range(2)]
                    for kc in range(nk):
                        for eh in range(2):
                            nc.tensor.matmul(
                                pso[eh][:], pt[:, kc],
                                v_h[:, kc, eh * 512:eh * 512 + 512],
                                start=(kc == 0), stop=(kc == nk - 1))

                    out_sb = opool.tile([P, D], fp16, name="out_sb",
                                        tag="out_sb")
                    nc.vector.tensor_copy(out_sb[:, :512], pso[0][:])
                    nc.scalar.copy(out_sb[:, 512:], pso[1][:])
                    nc.sync.dma_start(o_d[gb * P:(gb + 1) * P, :], out_sb[:])

                # one DMA for all row sums at the end
                nc.sync.dma_start(sums_d[:, :], sums_all[:])

    nc.compile()
    return nc


def _get_nc():
    if "nc" not in _CACHE:
        _CACHE["nc"] = _build_nc()
    return _CACHE["nc"]


def _shard_inputs(inputs_for_keys, inputs_for_values, inputs_for_queries,
                  WK, WQ, WV):
    xk = np.asarray(inputs_for_keys, dtype=np.float16)
    xv = np.asarray(inputs_for_values, dtype=np.float16)
    xq = np.asarray(inputs_for_queries, dtype=np.float16)
    wk = np.ascontiguousarray(np.asarray(WK, dtype=np.float16))
    wq = np.ascontiguousarray(np.asarray(WQ, dtype=np.float16))
    wv = np.ascontiguousarray(np.asarray(WV, dtype=np.float16))
    tri = np.triu(np.full((P, P), MASKV, np.float32), 1)  # mask k > q
    zero = np.zeros((P, P), np.float32)
    full = np.full((P, P), MASKV, np.float32)
    in_maps = []
    for c in range(NCORES):
        # pair-mates sit 4 apart (cross-die) so the AllGather rides the
        # D2D links; core c handles (batch c%4, key-parity c//4)
        b, h = c % 4, c // 4
        # key columns owned by this core: tiles h, h+2, ..., packed densely
        xkT = xk[b].T.reshape(D, SB, P)[:, h::2].reshape(D, KL)
        xvT = xv[b].T.reshape(D, SB, P)[:, h::2].reshape(D, KL)
        in_maps.append({
            "xk": np.ascontiguousarray(xkT),
            "xv": np.ascontiguousarray(xvT),
            "xq": np.ascontiguousarray(xq[b].T),
            "wk": wk,
            "wq": np.ascontiguousarray(wq[:, h * (D // 2):
                                          (h + 1) * (D // 2)]),
            "wv": wv,
            # last-local-tile mask for even/odd query blocks (see docstring)
            "mke": tri if h == 0 else full,
            "mko": zero if h == 0 else tri,
        })
    return in_maps


def _assemble(results):
    out = np.empty((B, S, D), dtype=np.float32)
    for b in range(B):
        r0, r1 = results[b], results[b + 4]
        o = r0["o"].astype(np.float32) + r1["o"].astype(np.float32)
        # sums arrive as [P, SB]: row q of block gb sits at [q, gb]
        s = (r0["sums"] + r1["sums"]).T.reshape(S)
        out[b] = o / s[:, None]
    return out


def _run(in_maps, **kwargs):
    from concourse.bass_utils import run_bass_kernel_spmd
    nc = _get_nc()
    return run_bass_kernel_spmd(nc, in_maps, list(range(NCORES)), **kwargs)


def kernel(inputs_for_keys, inputs_for_values, inputs_for_queries,
           WK, WQ, WV):
    in_maps = _shard_inputs(inputs_for_keys, inputs_for_values,
                            inputs_for_queries, WK, WQ, WV)
    res = _run(in_maps)
    return _assemble(res.results)


# revision 36
# speedup vs baseline: 1.4270x; 1.4270x over previous
"""Trainium2 Bass kernel: single-head causal attention (B=4, S=2048, D=1024).

reference:
  K = Xk @ WK; Q = Xq @ WQ; V = Xv @ WV          [B,S,D] @ [D,D]
  out = softmax(causal(Q K^T / sqrt(D))) @ V      [B,S,D]

Sharding over 8 NeuronCores (one SPMD program, no collectives):
  core c -> (batch b = c//2, key-parity h = c%2)
  Data-parallel over batch; within each pair, flash-attention-style split
  over KEYS: core h owns the key tiles {t : t % 2 == h} (the host feeds
  Xk/Xv columns for those keys, packed densely).  Each core projects
  K^T/V only for its own keys (split for free), projects Q fully
  (duplicated - every query row needs scores against both key subsets),
  and computes the UNNORMALIZED partial attention
      o_h[q, :] = sum_{k in keys_h, k <= q} exp(s_qk) V[k, :]
      s_h[q]    = sum_{k in keys_h, k <= q} exp(s_qk)
  The host combines the pair: out = (o_0 + o_1) / (s_0 + s_1).

Causal masking stays SPMD-uniform: for query block gb only the LAST local
key tile ever needs masking, and the needed pattern depends only on
(parity of gb, core parity).  Two host-fed [128,128] mask tiles
(mask_even / mask_odd) cover all cases: triangular for the core that owns
the diagonal tile, all -1e30 for the phantom tile the other core computes
(its exp underflows to exactly 0, contributing nothing), zeros when the
last local tile is fully visible.

The host feeds X pre-transposed ([D, S] layout) and pre-cast to fp16.

Per-core pipeline (fp16 matmuls on the PE, fp32 PSUM + fp32 softmax):
  Phase A: load X^T chunks + W (fp16, direct DMA);
           Q^T full -> [e, q], K^T (own keys) -> [e, k_loc],
           V (own keys) -> [k_loc, e] natural.
  Phase B: per 128-query block gb (big/small interleave, ending small):
           scores = Q^T.T K^T over the <= 8 local key tiles,
           data-driven mask on the last tile, p = exp(scores/sqrt(D)) on
           ACT with fp32 row sums (no max-shift: scaled logits ~N(0,0.33)),
           PE-transpose p tiles, o = p^T.T @ V accumulated over key tiles
           into two 512-wide PSUM banks, DMA out unnormalized (fp16) plus
           row sums (fp32).
"""
import numpy as np

B, S, D = 4, 2048, 1024
P = 128
SB = S // P            # 16 key/query blocks
DC = D // P            # 8 contraction chunks of 128
EB = D // P            # 8 e-blocks of 128
KL = S // 2            # 1024: per-core local key count
KLB = KL // P          # 8 local key tiles
INV_SQRT_D = float(1.0 / np.sqrt(np.float64(D)))
NCORES = 8
MASKV = -1e30

_CACHE = {}


def _build_nc():
    import concourse.bacc as bacc
    import concourse.mybir as mybir
    import concourse.tile as tile
    from concourse.masks import make_identity
    from contextlib import ExitStack

    fp32 = mybir.dt.float32
    fp16 = mybir.dt.float16
    Exp = mybir.ActivationFunctionType.Exp
    Add = mybir.AluOpType.add
    X = mybir.AxisListType.X

    nc = bacc.Bacc("TRN2", target_bir_lowering=False, debug=False,
                   num_devices=NCORES)

    xk_d = nc.dram_tensor("xk", [D, KL], fp16, kind="ExternalInput")
    xv_d = nc.dram_tensor("xv", [D, KL], fp16, kind="ExternalInput")
    xq_d = nc.dram_tensor("xq", [D, S], fp16, kind="ExternalInput")
    wk_d = nc.dram_tensor("wk", [D, D], fp16, kind="ExternalInput")
    # Q projection is pair-deduped: each core computes its e-half of Q and
    # the halves are exchanged with a background AllGather
    wq_d = nc.dram_tensor("wq", [D, D // 2], fp16, kind="ExternalInput")
    wv_d = nc.dram_tensor("wv", [D, D], fp16, kind="ExternalInput")
    mke_d = nc.dram_tensor("mke", [P, P], fp32, kind="ExternalInput")
    mko_d = nc.dram_tensor("mko", [P, P], fp32, kind="ExternalInput")
    o_d = nc.dram_tensor("o", [S, D], fp16, kind="ExternalOutput")
    sums_d = nc.dram_tensor("sums", [P, SB], fp32, kind="ExternalOutput")

    copy_ctr = [0]

    with tile.TileContext(nc) as tc:
        with ExitStack() as top:
            persist = top.enter_context(tc.tile_pool(name="persist", bufs=1))
            qt_h = persist.tile([P, EB, S], fp16, name="qt_h")
            kt_h = persist.tile([P, EB, KL], fp16, name="kt_h")
            v_h = persist.tile([P, KLB, D], fp16, name="v_h")
            ident16 = persist.tile([P, P], fp16, name="ident16")
            mke = persist.tile([P, P], fp32, name="mke")
            mko = persist.tile([P, P], fp32, name="mko")
            sums_all = persist.tile([P, SB], fp32, name="sums_all")

            def alt_copy(dst, src):
                # round-robin PSUM->SBUF copies 2:1 between DVE and ACT
                i = copy_ctr[0]
                copy_ctr[0] += 1
                if i % 3 == 2:
                    nc.scalar.copy(dst, src)
                else:
                    nc.vector.tensor_copy(dst, src)

            # ---------------- Phase A: projections ----------------
            with ExitStack() as pa:
                wpool = pa.enter_context(tc.tile_pool(name="wpool", bufs=1))
                xtpool = pa.enter_context(tc.tile_pool(name="xtpool",
                                                       bufs=8))
                dram = pa.enter_context(
                    tc.tile_pool(name="dram", bufs=1, space="DRAM"))
                psA = pa.enter_context(
                    tc.tile_pool(name="psA", bufs=3, space="PSUM"))

                # masks/identity first: tiny DMAs + iota, no load deps
                make_identity(nc, ident16[:])
                nc.sync.dma_start(mke[:], mke_d[:, :])
                nc.sync.dma_start(mko[:], mko_d[:, :])

                qs = [nc.gpsimd, nc.sync, nc.scalar]

                def load_w(w_d, nm, qoff, ecols=D, q=None):
                    wh = wpool.tile([P, DC, ecols], fp16, name=nm, tag=nm)
                    src = w_d.rearrange("(c p) e -> p c e", p=P)
                    for i in range(4):
                        (q or qs[(qoff + i) % 3]).dma_start(
                            wh[:, 2 * i:2 * i + 2], src[:, 2 * i:2 * i + 2])
                    return wh

                def load_xt(x_d, ch, qoff, q=None):
                    """Columns [ch*512, (ch+1)*512) of x^T [D, cols] (fp16):
                    two parallel half-loads -> [P(d), DC, 512] fp16."""
                    xt = xtpool.tile([P, DC, 512], fp16, name="xt", tag="xt")
                    src = x_d.rearrange("(c p) s -> p c s", p=P)[
                        :, :, ch * 512:(ch + 1) * 512]
                    (q or qs[qoff % 3]).dma_start(xt[:, :4], src[:, :4])
                    (q or qs[(qoff + 1) % 3]).dma_start(xt[:, 4:], src[:, 4:])
                    return xt

                def blocker(region):
                    # a tiny sync-queue DMA reading an SBUF region that
                    # compute only writes later: its $S-wait stalls the
                    # sync queue, so prefetch triggers emitted after it
                    # genuinely hold off until that compute lands --
                    # without it every prefetch fires at t~8us and the
                    # critical first 3MB crawls behind 10MB of competing
                    # traffic
                    j = dram.tile([P, 64], fp16, name="junk")
                    nc.sync.dma_start(j[:], region)

                # Prefetch everything in consumption order: the 3 W's and
                # all 8 X chunks fit in wpool(3)+xtpool(8) simultaneously,
                # so no matmul after the first ever waits on a load.  The
                # first-needed 3MB (xq chunk 0 + wq) goes out first so the
                # first matmul group isn't stuck behind competing traffic.
                # critical first batch: xq chunk 0 on gpsimd+scalar, wq
                # quarters spread over all three queues
                xq_t = [load_xt(xq_d, 0, 0)]
                wq_h = load_w(wq_d, "w_q", 1, ecols=D // 2)

                # warm the PE's HAM clock gate with throwaway matmuls on the
                # identity tile while the first loads are still in flight
                for _ in range(6):
                    wps = psA.tile([P, 512], fp32, name="warm", tag="psa")
                    for j in range(8):
                        nc.tensor.matmul(wps[:, :P], ident16[:], ident16[:],
                                         start=(j == 0), stop=(j == 7))

                NEB = EB // 2  # local e-blocks of the deduped Q projection

                # Q-half projection, then K (own keys): [e, s] W-stationary.
                # Later loads are anchored behind earlier chunks' results
                # so the critical first 3MB has the DMA engines to itself;
                # everything else still arrives well ahead of use.
                wk_h = wv_h = None
                xk_t, xv_t = [], []
                for w_i, (dst, ncols, neb) in enumerate(
                        ((qt_h, S, NEB), (kt_h, KL, EB))):
                    w_h = wq_h if w_i == 0 else wk_h
                    for ch in range(ncols // 512):
                        xt = (xq_t if w_i == 0 else xk_t)[ch]
                        for eb in range(neb):
                            ps = psA.tile([P, 512], fp32, name="psa",
                                          tag="psa")
                            for dc in range(DC):
                                nc.tensor.matmul(
                                    ps[:],
                                    w_h[:, dc, eb * P:(eb + 1) * P],
                                    xt[:, dc, :],
                                    start=(dc == 0), stop=(dc == DC - 1))
                            alt_copy(dst[:, eb, ch * 512:ch * 512 + 512],
                                     ps[:])
                        if w_i == 0 and ch == 0:
                            # deferred prefetch of everything else, held
                            # back behind the first projected result
                            blocker(qt_h[:, 0, 0:64])
                            xq_t += [load_xt(xq_d, c, 0, q=nc.sync)
                                     for c in (1, 2, 3)]
                            wk_h = load_w(wk_d, "w_k", 0, q=nc.sync)
                            xk_t += [load_xt(xk_d, c, 0, q=nc.sync)
                                     for c in (0, 1)]
                            wv_h = load_w(wv_d, "w_v", 0, q=nc.sync)
                            xv_t += [load_xt(xv_d, c, 0, q=nc.sync)
                                     for c in (0, 1)]
                        if w_i == 0 and ch == 3:
                            # pair AllGather of the Q e-halves, hidden
                            # under the K and V projections; rank r's half
                            # lands at e-blocks [r*NEB, (r+1)*NEB) which
                            # matches how the host sliced WQ
                            in_b = dram.tile([P, NEB, S], fp16, name="q_in")
                            out_b = dram.tile([2, P, NEB, S], fp16,
                                              name="q_out")
                            nc.scalar.dma_start(in_b[:], qt_h[:, :NEB])
                            nc.gpsimd.collective_compute(
                                "AllGather",
                                mybir.AluOpType.bypass,
                                replica_groups=[[0, 1], [2, 3],
                                                [4, 5], [6, 7]],
                                ins=[in_b.opt()],
                                outs=[out_b.opt()],
                            )
                            nc.sync.dma_start(qt_h[:, :NEB], out_b[0])
                            nc.sync.dma_start(qt_h[:, NEB:], out_b[1])

                # V projection (own keys, full e): out[k, e] X^T-stationary
                for ch in range(KL // 512):
                    xt = xv_t[ch]
                    for a in range(4):
                        for eh in range(2):
                            ps = psA.tile([P, 512], fp32, name="psa",
                                          tag="psa")
                            for dc in range(DC):
                                nc.tensor.matmul(
                                    ps[:],
                                    xt[:, dc, a * P:(a + 1) * P],
                                    wv_h[:, dc, eh * 512:eh * 512 + 512],
                                    start=(dc == 0), stop=(dc == DC - 1))
                            alt_copy(
                                v_h[:, ch * 4 + a,
                                    eh * 512:eh * 512 + 512], ps[:])

            # ---------------- Phase B: causal attention ----------------
            with ExitStack() as pb:
                ppool = pb.enter_context(tc.tile_pool(name="ppool", bufs=3))
                ptpool = pb.enter_context(tc.tile_pool(name="ptpool", bufs=3))
                smpool = pb.enter_context(tc.tile_pool(name="smpool", bufs=4))
                opool = pb.enter_context(tc.tile_pool(name="opool", bufs=4))
                psBs = pb.enter_context(
                    tc.tile_pool(name="psBs", bufs=2, space="PSUM"))
                psBt = pb.enter_context(
                    tc.tile_pool(name="psBt", bufs=2, space="PSUM"))
                psBo = pb.enter_context(
                    tc.tile_pool(name="psBo", bufs=2, space="PSUM"))

                # big/small interleave: every small block's serial softmax
                # chain hides behind a big block's matmul stream; end with
                # the smallest block so the tail is minimal
                order = []
                for i in range(SB // 2 - 1):
                    order.append(SB - 1 - i)
                    order.append(i + 1)
                order += [SB // 2, 0]
                for gb in order:
                    nk = gb // 2 + 1   # local key tiles (incl. any phantom)
                    kw = nk * P        # local visible key width
                    nch = (kw + 511) // 512
                    mask = mke if gb % 2 == 0 else mko

                    # streaming softmax without max-shift: scaled logits are
                    # ~N(0,0.33), so exp(s/sqrt(D)) is safely inside fp32
                    # range and softmax is shift-invariant. Each QK chunk
                    # goes straight from PSUM through exp; normalization
                    # happens on the host after the pair-combine.
                    p16 = ppool.tile([P, KL], fp16, name="p16", tag="p16")
                    sums4 = smpool.tile([P, 2], fp32, name="sums4",
                                        tag="sums4")
                    pt = ptpool.tile([P, KLB, P], fp16, name="pt", tag="pt")
                    for ci in range(nch):
                        c0 = ci * 512
                        w = min(512, kw - c0)
                        ps = psBs.tile([P, 512], fp32, name="ps_s", tag="ps_s")
                        for dc in range(DC):
                            nc.tensor.matmul(
                                ps[:, :w],
                                qt_h[:, dc, gb * P:(gb + 1) * P],
                                kt_h[:, dc, c0:c0 + w],
                                start=(dc == 0), stop=(dc == DC - 1))
                        if c0 + w == kw:
                            # data-driven causal mask on the last local tile
                            nc.vector.tensor_tensor(
                                ps[:, w - P:w], ps[:, w - P:w], mask[:], Add)
                        nc.scalar.activation(p16[:, c0:c0 + w], ps[:, :w],
                                             Exp, bias=0.0, scale=INV_SQRT_D,
                                             accum_out=sums4[:, ci:ci + 1])
                        for k0 in range(c0 // P, c0 // P + w // P, 4):
                            kn = min(4, nk - k0)
                            pst = psBt.tile([P, 512], fp16, name="ps_t",
                                            tag="ps_t")
                            for j in range(kn):
                                nc.tensor.transpose(
                                    pst[:, j * P:(j + 1) * P],
                                    p16[:, (k0 + j) * P:(k0 + j + 1) * P],
                                    ident16[:])
                            nc.vector.tensor_copy(
                                pt[:, k0:k0 + kn], pst[:, :kn * P])

                    nc.vector.tensor_reduce(sums_all[:, gb:gb + 1],
                                            sums4[:, :nch], X, Add)

                    pso = [psBo.tile([P, 512], fp32, name=f"ps_o{eh}",
                                     tag=f"ps_o{eh}") for eh in range(2)]
                    for kc in range(nk):
                        for eh in range(2):
                            nc.tensor.matmul(
                                pso[eh][:], pt[:, kc],
                                v_h[:, kc, eh * 512:eh * 512 + 512],
                                start=(kc == 0), stop=(kc == nk - 1))

                    out_sb = opool.tile([P, D], fp16, name="out_sb",
                                        tag="out_sb")
                    nc.vector.tensor_copy(out_sb[:, :512], pso[0][:])
                    nc.scalar.copy(out_sb[:, 512:], pso[1][:])
                    nc.sync.dma_start(o_d[gb * P:(gb + 1) * P, :], out_sb[:])

                # one DMA for all row sums at the end
                nc.sync.dma_start(sums_d[:, :], sums_all[:])

    nc.compile()
    return nc


def _get_nc():
    if "nc" not in _CACHE:
        _CACHE["nc"] = _build_nc()
    return _CACHE["nc"]


def _shard_inputs(inputs_for_keys, inputs_for_values, inputs_for_queries,
                  WK, WQ, WV):
    xk = np.asarray(inputs_for_keys, dtype=np.float16)
    xv = np.asarray(inputs_for_values, dtype=np.float16)
    xq = np.asarray(inputs_for_queries, dtype=np.float16)
    wk = np.ascontiguousarray(np.asarray(WK, dtype=np.float16))
    wq = np.ascontiguousarray(np.asarray(WQ, dtype=np.float16))
    wv = np.ascontiguousarray(np.asarray(WV, dtype=np.float16))
    tri = np.triu(np.full((P, P), MASKV, np.float32), 1)  # mask k > q
    zero = np.zeros((P, P), np.float32)
    full = np.full((P, P), MASKV, np.float32)
    in_maps = []
    for c in range(NCORES):
        b, h = divmod(c, 2)
        # key columns owned by this core: tiles h, h+2, ..., packed densely
        xkT = xk[b].T.reshape(D, SB, P)[:, h::2].reshape(D, KL)
        xvT = xv[b].T.reshape(D, SB, P)[:, h::2].reshape(D, KL)
        in_maps.append({
            "xk": np.ascontiguousarray(xkT),
            "xv": np.ascontiguousarray(xvT),
            "xq": np.ascontiguousarray(xq[b].T),
            "wk": wk,
            "wq": np.ascontiguousarray(wq[:, h * (D // 2):
                                          (h + 1) * (D // 2)]),
            "wv": wv,
            # last-local-tile mask for even/odd query blocks (see docstring)
            "mke": tri if h == 0 else full,
            "mko": zero if h == 0 else tri,
        })
    return in_maps


def _assemble(results):
    out = np.empty((B, S, D), dtype=np.float32)
    for b in range(B):
        r0, r1 = results[2 * b], results[2 * b + 1]
        o = r0["o"].astype(np.float32) + r1["o"].astype(np.float32)
        # sums arrive as [P, SB]: row q of block gb sits at [q, gb]
        s = (r0["sums"] + r1["sums"]).T.reshape(S)
        out[b] = o / s[:, None]
    return out


def _run(in_maps, **kwargs):
    from concourse.bass_utils import run_bass_kernel_spmd
    nc = _get_nc()
    return run_bass_kernel_spmd(nc, in_maps, list(range(NCORES)), **kwargs)


def kernel(inputs_for_keys, inputs_for_values, inputs_for_queries,
           WK, WQ, WV):
    in_maps = _shard_inputs(inputs_for_keys, inputs_for_values,
                            inputs_for_queries, WK, WQ, WV)
    res = _run(in_maps)
    return _assemble(res.results)
